# revision 10
# baseline (speedup 1.0000x reference)
"""Two-layer GCN (PyG GCNConv x2 + rrelu) on 8 Trainium2 NeuronCores.

Math: with A = adjacency-with-multiplicity + I (self loops), deg = in-degree,
dinv = deg^-1/2:
    z1[v] = dinv[v] * (sum_{u->v} dinv[u]*x[u]) @ W1 + b1
    g[u]  = dinv[u] * rrelu(z1[u])                      (dinv pre-folded for L2)
    z2[v] = dinv[v] * (sum_{u->v} g[u]) @ W2 + b2
(aggregation is linear, so the dense W matmul is applied post-aggregation on
the [128, 128] per-destination-block aggregate -- 128x less PE work than
transforming every edge message).

Sharding: destinations are range-sharded across the 8 cores (12544 nodes
each).  Every core holds a full replicated copy of the (dinv-prescaled,
bf16) source-feature table in its own HBM and gathers the source rows for
its edges with dma_gather.  Per destination block of 128 nodes, gathered
edge-message chunks [128 edges, 128 feat] are scatter-reduced on the
TensorEngine by multiplying with a one-hot selector Sel[e, dest] =
(d[e] == dest), generated on-device with a single is_equal tensor_tensor op
per chunk group.  Two NEFF dispatches (layer 1, layer 2); the host only
transposes/concats the bf16 activations between them.

The harness calls kernel(**inputs) with the full inputs; everything below
(index bucketing, program build, compile, SPMD run on cores 0-7, unshard)
happens inside.
"""

import sys

for _p in ("/opt/trn_rl_repo",):
    if _p not in sys.path:
        sys.path.insert(0, _p)

import numpy as np
import ml_dtypes

import concourse.bacc as bacc
import concourse.bass as bass
import concourse.mybir as mybir
import concourse.tile as tile
from concourse.bass_utils import run_bass_kernel_spmd

P = 128  # partition width == dest block width == feature width
RRELU_SLOPE = (1.0 / 8.0 + 1.0 / 3.0) / 2.0


class Cfg:
    def __init__(self, n_nodes, n_cores, blocks_per_core, superblock, in_f,
                 out1_f, out2_f, src_window=32768):
        self.n_nodes = n_nodes
        self.n_cores = n_cores
        self.bpc = blocks_per_core            # dest blocks per core
        self.sb = superblock                  # blocks per superblock
        assert blocks_per_core % superblock == 0
        self.sb_count = blocks_per_core // superblock
        self.in_f = in_f                      # 128
        self.out1_f = out1_f                  # 128
        self.out2_f = out2_f                  # 64
        self.src_window = src_window          # int16 gather range per window
        self.nodes_per_core = blocks_per_core * P
        self.n_pad = n_cores * self.nodes_per_core
        assert self.n_pad >= n_nodes
        self.n_chunks = -(-self.n_pad // src_window)
        self.tab_rows = self.n_chunks * src_window


# src_window = n_pad/4 gives four EVEN gather windows (fits int16, %128==0)
FULL = Cfg(n_nodes=100000, n_cores=8, blocks_per_core=98, superblock=7,
           in_f=128, out1_f=128, out2_f=64, src_window=25088)

# One dma_gather call is limited to 63 data descriptors per SDMA engine
# (64-desc packet/ring bound, hit empirically at num_idxs=1024), so calls
# are split into groups of <= MAX_CALL_COLS columns of 128 indices.
MAX_CALL_COLS = 7


def _call_plan(caps):
    """Deterministic per-block gather call layout: list of
    (chunk k, col offset within the (block,k) cap range, n_cols)."""
    plan = []
    for k, cap in enumerate(caps):
        c0 = 0
        while c0 < cap:
            n = min(MAX_CALL_COLS, cap - c0)
            plan.append((k, c0, n))
            c0 += n
    return plan


# --------------------------------------------------------------------------
# host-side index preprocessing
# --------------------------------------------------------------------------

def preprocess(edge_index, cfg):
    """Bucket edges (plus self loops) by (dest core, dest block, src window);
    build per-core gather index / dest-local tables and the degree scaling."""
    row = edge_index[0].astype(np.int64)
    col = edge_index[1].astype(np.int64)
    n = cfg.n_nodes

    deg = np.bincount(col, minlength=cfg.n_pad).astype(np.float64) + 1.0
    dinv = (1.0 / np.sqrt(deg)).astype(np.float32)
    dinv[n:] = 1.0

    # append self loops (real nodes only)
    loops = np.arange(n, dtype=np.int64)
    row = np.concatenate([row, loops])
    col = np.concatenate([col, loops])

    # order edges by (dest block, src window) once, globally
    blk = col >> 7                      # global dest block
    chunk = row // cfg.src_window
    order = np.lexsort((chunk, blk))
    row, col, blk, chunk = row[order], col[order], blk[order], chunk[order]

    n_blocks = cfg.n_cores * cfg.bpc
    # counts[block, chunk]
    counts = np.zeros((n_blocks, cfg.n_chunks), dtype=np.int64)
    np.add.at(counts, (blk, chunk), 1)

    caps = np.maximum(-(-counts.max(axis=0) // P), 1)  # columns per chunk
    if cfg is FULL:
        caps = np.maximum(caps, 6)  # stabilize program shape across datasets
    caps = caps.astype(np.int64)
    c_total = int(caps.sum())

    # per (block, chunk) slice starts in the sorted edge array
    bc_start = np.zeros(n_blocks * cfg.n_chunks + 1, dtype=np.int64)
    np.cumsum(counts.reshape(-1), out=bc_start[1:])

    sb_cols = cfg.sb * c_total                     # supertile columns
    colbase = np.concatenate([[0], np.cumsum(cfg.sb * caps)])[:-1]  # per chunk

    plan = _call_plan([int(x) for x in caps])
    per_core = []
    for c in range(cfg.n_cores):
        idx_parts = []                                   # int16, per call
        d_tab = np.full((P, cfg.bpc * c_total), -1.0, dtype=np.float64)
        for s in range(cfg.sb_count):
            for b7 in range(cfg.sb):
                b_loc = s * cfg.sb + b7
                b_glob = c * cfg.bpc + b_loc
                segs = []
                for k in range(cfg.n_chunks):
                    cap = int(caps[k])
                    lo = bc_start[b_glob * cfg.n_chunks + k]
                    hi = bc_start[b_glob * cfg.n_chunks + k + 1]
                    cnt = hi - lo
                    assert cnt <= cap * P, (cnt, cap * P)
                    seg = np.zeros(cap * P, dtype=np.int64)
                    seg[:cnt] = row[lo:hi] - k * cfg.src_window
                    # duplicate-pad with a harmless valid index (d stays -1)
                    if cnt < cap * P:
                        seg[cnt:] = seg[0] if cnt > 0 else 0
                    assert seg.min() >= 0 and seg.max() < cfg.src_window
                    segs.append(seg)
                    # dest-local ids for this block's columns of the supertile
                    gcol0 = s * sb_cols + colbase[k] + b7 * cap
                    d_seg = np.full(cap * P, -1.0)
                    d_seg[:cnt] = (col[lo:hi] - b_glob * P).astype(np.float64)
                    d_tab[:, gcol0:gcol0 + cap] = d_seg.reshape(cap, P).T
                for (k, c0, ncols) in plan:
                    idx_parts.append(
                        segs[k][c0 * P:(c0 + ncols) * P].astype(np.int16))
        idx_flat = [a.reshape(-1, 16).T for a in idx_parts]   # [16, n/16] each
        idx_tab = np.concatenate(idx_flat, axis=1)
        idx_tab = np.tile(idx_tab, (8, 1))                    # [128, total/16]
        per_core.append({
            "idx_tab": np.ascontiguousarray(idx_tab),
            "d_tab": np.ascontiguousarray(d_tab.astype(ml_dtypes.bfloat16)),
            "dinv_sl": np.ascontiguousarray(
                dinv[c * cfg.nodes_per_core:(c + 1) * cfg.nodes_per_core]
            ).reshape(1, -1),
        })

    meta = {
        "caps": caps, "c_total": c_total, "colbase": colbase,
        "sb_cols": sb_cols, "dinv": dinv, "per_core": per_core,
    }
    return meta


# --------------------------------------------------------------------------
# bass program (one GCN layer, SPMD across cores; all data via inputs)
# --------------------------------------------------------------------------

def build_layer_program(cfg, caps, layer):
    """layer=1: out = bf16 gs1T [128, nodes_per_core]  (dinv*rrelu(z1), F-major)
       layer=2: out = f32  z2T  [out2_f, nodes_per_core]"""
    caps = [int(x) for x in caps]
    c_total = sum(caps)
    sb_cols = cfg.sb * c_total
    colbase = np.concatenate([[0], np.cumsum([cfg.sb * k for k in caps])])[:-1]
    plan = _call_plan(caps)
    out_f = cfg.out1_f if layer == 1 else cfg.out2_f
    out_dt = mybir.dt.bfloat16 if layer == 1 else mybir.dt.float32
    idx_cols_sb = sb_cols * P // 16          # idx free-dim per superblock
    G = 8                                     # sel-gen chunk group width

    nc = bacc.Bacc("TRN2", target_bir_lowering=False, debug=False,
                   num_devices=cfg.n_cores,
                   num_swdge_queues=min(4, cfg.n_chunks))
    dt = mybir.dt
    src_tab = nc.dram_tensor("src_tab", [cfg.tab_rows, P], dt.bfloat16,
                             kind="ExternalInput")
    w_in = nc.dram_tensor("w", [P, out_f], dt.bfloat16, kind="ExternalInput")
    bias_in = nc.dram_tensor("bias", [out_f, 1], dt.float32, kind="ExternalInput")
    dinv_in = nc.dram_tensor("dinv_sl", [1, cfg.nodes_per_core], dt.float32,
                             kind="ExternalInput")
    idx_in = nc.dram_tensor("idx_tab", [P, cfg.sb_count * idx_cols_sb], dt.int16,
                            kind="ExternalInput")
    d_in = nc.dram_tensor("d_tab", [P, cfg.bpc * c_total], dt.bfloat16,
                          kind="ExternalInput")
    iota_in = nc.dram_tensor("iota", [P, G * P], dt.bfloat16, kind="ExternalInput")
    ones_in = nc.dram_tensor("ones", [1, P], dt.float32, kind="ExternalInput")
    out_t = nc.dram_tensor("out_t", [out_f, cfg.nodes_per_core], out_dt,
                           kind="ExternalOutput")

    with tile.TileContext(nc) as tc:
        with (
            tc.tile_pool(name="const", bufs=1) as const_pool,
            tc.tile_pool(name="idx", bufs=2) as idx_pool,
            tc.tile_pool(name="msg", bufs=2) as msg_pool,
            tc.tile_pool(name="sel", bufs=6) as sel_pool,
            tc.tile_pool(name="aggsb", bufs=3) as aggsb_pool,
            tc.tile_pool(name="tmp", bufs=3) as tmp_pool,
            tc.tile_pool(name="outsb", bufs=2) as out_pool,
            tc.tile_pool(name="psA", bufs=2, space="PSUM") as agg_psum,
            tc.tile_pool(name="psZ", bufs=2, space="PSUM") as z_psum,
            tc.tile_pool(name="psD", bufs=2, space="PSUM") as d_psum,
        ):
            w_sb = const_pool.tile([P, out_f], dt.bfloat16)
            nc.sync.dma_start(out=w_sb[:], in_=w_in[:])
            bias_sb = const_pool.tile([out_f, 1], dt.float32)
            nc.sync.dma_start(out=bias_sb[:], in_=bias_in[:])
            dinv_sb = const_pool.tile([1, cfg.nodes_per_core], dt.float32)
            nc.sync.dma_start(out=dinv_sb[:], in_=dinv_in[:])
            iota_sb = const_pool.tile([P, G * P], dt.bfloat16)
            nc.sync.dma_start(out=iota_sb[:], in_=iota_in[:])
            ones_sb = const_pool.tile([1, P], dt.float32)
            nc.sync.dma_start(out=ones_sb[:], in_=ones_in[:])
            d_sb = const_pool.tile([P, cfg.bpc * c_total], dt.bfloat16)
            nc.sync.dma_start(out=d_sb[:], in_=d_in[:])

            for s in range(cfg.sb_count):
                idx_sb = idx_pool.tile([P, idx_cols_sb], dt.int16)
                nc.sync.dma_start(
                    out=idx_sb[:],
                    in_=idx_in[:, s * idx_cols_sb:(s + 1) * idx_cols_sb])

                msg = msg_pool.tile([P, sb_cols, P], dt.bfloat16)
                off = 0
                for b7 in range(cfg.sb):
                    for (k, c0, ncols) in plan:
                        n_idx = ncols * P
                        mcol0 = colbase[k] + b7 * caps[k] + c0
                        nc.gpsimd.dma_gather(
                            msg[:, mcol0:mcol0 + ncols, :],
                            src_tab[k * cfg.src_window:
                                    (k + 1) * cfg.src_window, :],
                            idx_sb[:, off:off + n_idx // 16],
                            n_idx, n_idx, P,
                            queue_num=k % 4,
                        )
                        off += n_idx // 16

                out_sb = out_pool.tile([out_f, cfg.sb * P], out_dt)
                for b7 in range(cfg.sb):
                    b_loc = s * cfg.sb + b7
                    # one-hot selectors for this block's chunk columns
                    sels = []          # (sel_tile, local col, supertile col)
                    for k in range(cfg.n_chunks):
                        cap = caps[k]
                        # d_tab columns are supertile-global
                        dcol0 = s * sb_cols + colbase[k] + b7 * cap
                        done = 0
                        while done < cap:
                            g = min(G, cap - done)
                            sel = sel_pool.tile([P, G * P], dt.bfloat16)
                            nc.vector.tensor_tensor(
                                sel[:, :g * P],
                                iota_sb[:, :g * P],
                                d_sb[:, dcol0 + done:dcol0 + done + g]
                                    .to_broadcast([P, g, P]),
                                mybir.AluOpType.is_equal,
                            )
                            for j in range(g):
                                sels.append(
                                    (sel, j,
                                     colbase[k] + b7 * cap + done + j))
                            done += g

                    agg = agg_psum.tile([P, P], dt.float32)
                    n_ch = len(sels)
                    for ci, (sel, j, mcol) in enumerate(sels):
                        nc.tensor.matmul(
                            agg[:],
                            lhsT=msg[:, mcol, :],
                            rhs=sel[:, j * P:(j + 1) * P],
                            start=(ci == 0), stop=(ci == n_ch - 1),
                        )

                    # dinv broadcast tile for this block (rank-1 matmul into
                    # psum, then to SBUF -- DVE ops may read only one PSUM
                    # operand, and zps below is already in PSUM)
                    dps = d_psum.tile([P, P], dt.float32)
                    nc.tensor.matmul(
                        dps[:], lhsT=ones_sb[:],
                        rhs=dinv_sb[:, b_loc * P:(b_loc + 1) * P],
                        start=True, stop=True)
                    dbc = aggsb_pool.tile([P, P], dt.float32, tag="dbc")
                    nc.scalar.copy(dbc[:], dps[:])

                    aggsb = aggsb_pool.tile([P, P], dt.bfloat16, tag="aggsb")
                    nc.vector.tensor_copy(aggsb[:], agg[:])

                    zps = z_psum.tile([out_f, P], dt.float32)
                    nc.tensor.matmul(zps[:], lhsT=w_sb[:], rhs=aggsb[:],
                                     start=True, stop=True)

                    o_sl = out_sb[:, b7 * P:(b7 + 1) * P]
                    if layer == 1:
                        t1 = tmp_pool.tile([P, P], dt.float32, tag="t1")
                        nc.vector.tensor_tensor(t1[:], zps[:], dbc[:],
                                                mybir.AluOpType.mult)
                        u = tmp_pool.tile([P, P], dt.float32, tag="u")
                        nc.vector.tensor_scalar_add(u[:], t1[:], bias_sb[:, 0:1])
                        rr = tmp_pool.tile([P, P], dt.float32, tag="rr")
                        nc.vector.scalar_tensor_tensor(
                            rr[:], u[:], float(RRELU_SLOPE), u[:],
                            mybir.AluOpType.mult, mybir.AluOpType.max)
                        nc.vector.tensor_tensor(o_sl, rr[:], dbc[:],
                                                mybir.AluOpType.mult)
                    else:
                        t1 = tmp_pool.tile([out_f, P], dt.float32, tag="t1")
                        nc.vector.tensor_tensor(t1[:], zps[:], dbc[:out_f, :],
                                                mybir.AluOpType.mult)
                        nc.vector.tensor_scalar_add(o_sl, t1[:], bias_sb[:, 0:1])

                nc.sync.dma_start(
                    out=out_t[:, s * cfg.sb * P:(s + 1) * cfg.sb * P],
                    in_=out_sb[:])

    nc.compile()
    return nc


# --------------------------------------------------------------------------
# orchestration
# --------------------------------------------------------------------------

def _iota_tile(G=8):
    return np.tile(np.arange(P, dtype=np.float32), G)[None, :].repeat(P, 0).astype(ml_dtypes.bfloat16)


def _run_gcn(x, edge_index, W1, b1, W2, b2, cfg, runner=None, want_times=False):
    """Shared driver; runner(nc, in_maps) -> list of per-core output dicts."""
    meta = preprocess(np.asarray(edge_index), cfg)
    dinv = meta["dinv"]
    npc = cfg.nodes_per_core

    if runner is None:
        times = []

        def runner(nc, in_maps):
            r = run_bass_kernel_spmd(nc, in_maps, core_ids=list(range(cfg.n_cores)),
                                     trace=want_times)
            if want_times:
                times.append(r.exec_time_ns)
            return r.results
    else:
        times = None

    x = np.asarray(x, dtype=np.float32)
    xs = np.zeros((cfg.tab_rows, P), dtype=ml_dtypes.bfloat16)
    xs[:cfg.n_nodes] = (x * dinv[:cfg.n_nodes, None]).astype(ml_dtypes.bfloat16)

    iota = _iota_tile()
    ones = np.ones((1, P), dtype=np.float32)
    w1 = np.asarray(W1, np.float32).astype(ml_dtypes.bfloat16)
    w2 = np.zeros((P, cfg.out2_f), dtype=ml_dtypes.bfloat16)
    w2[:] = np.asarray(W2, np.float32).astype(ml_dtypes.bfloat16)
    b1c = np.asarray(b1, np.float32).reshape(-1, 1)
    b2c = np.asarray(b2, np.float32).reshape(-1, 1)

    nc1 = build_layer_program(cfg, meta["caps"], layer=1)
    in_maps = [
        {"src_tab": xs, "w": w1, "bias": b1c, "iota": iota, "ones": ones,
         **{k: pc[k] for k in ("idx_tab", "d_tab", "dinv_sl")}}
        for pc in meta["per_core"]
    ]
    res1 = runner(nc1, in_maps)

    gs = np.zeros((cfg.tab_rows, P), dtype=ml_dtypes.bfloat16)
    for c in range(cfg.n_cores):
        gs[c * npc:(c + 1) * npc] = res1[c]["out_t"].T

    nc2 = build_layer_program(cfg, meta["caps"], layer=2)
    for c in range(cfg.n_cores):
        in_maps[c] = dict(in_maps[c])
        in_maps[c]["src_tab"] = gs
        in_maps[c]["w"] = w2
        in_maps[c]["bias"] = b2c
    res2 = runner(nc2, in_maps)

    out = np.zeros((cfg.n_pad, cfg.out2_f), dtype=np.float32)
    for c in range(cfg.n_cores):
        out[c * npc:(c + 1) * npc] = res2[c]["out_t"].T
    out = out[:cfg.n_nodes]
    if want_times and times is not None:
        return out, times
    return out


def kernel(x, edge_index, W1, b1, W2, b2):
    return _run_gcn(x, edge_index, W1, b1, W2, b2, FULL)
